# revision 1
# baseline (speedup 1.0000x reference)
"""Masked community-RNN kernel v2 — transposed formulation, 8 NeuronCores.

Model (T=100, B=128, H=2048, 4 modules of 512):
    h_t = tanh(x_t @ Wih.T + b_ih + h_{t-1} @ Whh.T + b_hh)
    out_t = h_t @ Wout.T + b_out   -> split into 4 modules

This environment charges a ~flat ~35-50us per engine instruction, so the
design minimizes per-core critical-path instruction count.

Transposed matmul formulation: PSUM accumulates z.T [batch 128, h_slice 256]
with stationary = hT chunks [h_in 128, b 128] (the gathered hall buffer,
identical columns on every core) and moving = per-core Whh.T tiles
[h_in 128, h_out 256] (N=256 wide).  Per core per step:
  - 2 opener matmuls (input projection, K=65/64; bias rides an ones-row)
  - 16 recurrent matmuls (one per hall chunk)
  - 2 PE transposes (h' [b,256] -> hT chunks for the next step)
  - 1 tanh (scalar), 1 PSUM->SBUF copy (scalar), 1 broadcast (gpsimd)
20 PE instructions/step vs 42 in the classic formulation.

Readout is fully off the critical path: every core DMAs the gathered
h [128, 2048] to DRAM each step; a batched tail (per-core blocks of 4
steps, N=512 readout matmuls) computes out = h @ Wout.T + b_out.
"""

import numpy as np
import ml_dtypes

import concourse.bass as bass
import concourse.bacc as bacc
import concourse.mybir as mybir
from concourse.bass_utils import run_bass_kernel_spmd

BF16 = ml_dtypes.bfloat16

NCORES = 8
T_FULL = 100
B = 128
H = 2048
HS = 256          # h_out slice per core
NK = 16           # hall chunks
XB = 4            # steps per x batch
OUT_TOT = 64
NBLK = 25         # readout blocks of 4 steps
NROUND = 4        # tail rounds per core


def build(T=T_FULL, t_io=None):
    t_io = t_io or T
    nxb = T // XB
    nxb_io = t_io // XB
    nc = bacc.Bacc(
        num_devices=NCORES,
        target_bir_lowering=False,
        dynamic_dma_scratch_size=65536,
    )
    f32, bf16 = mybir.dt.float32, mybir.dt.bfloat16

    # per-core inputs
    wht = nc.dram_tensor("wht", [128, NK * HS], bf16, kind="ExternalInput")
    wihA = nc.dram_tensor("wihA", [65, HS], bf16, kind="ExternalInput")
    wihB = nc.dram_tensor("wihB", [64, HS], bf16, kind="ExternalInput")
    wout = nc.dram_tensor("wout", [128, NK * OUT_TOT], bf16, kind="ExternalInput")
    bout = nc.dram_tensor("bout", [OUT_TOT, 1], f32, kind="ExternalInput")
    ident = nc.dram_tensor("ident", [128, 128], bf16, kind="ExternalInput")
    myoff = nc.dram_tensor("myoff", [1, 1], mybir.dt.uint32, kind="ExternalInput")
    tblk = nc.dram_tensor("tblk", [1, NROUND], mybir.dt.uint32, kind="ExternalInput")
    xt = nc.dram_tensor("xt", [nxb_io, 128, XB * 128], bf16, kind="ExternalInput")
    out_tail = nc.dram_tensor("out_tail", [NROUND, OUT_TOT, 512], f32,
                              kind="ExternalOutput")
    hstore = nc.dram_tensor("hstore", [t_io, 128, H], bf16, kind="Internal")

    import contextlib
    with contextlib.ExitStack() as stk:
        E = stk.enter_context
        w_sem = E(nc.semaphore("w_sem"))
        x_sem = E(nc.semaphore("x_sem"))
        recv = [E(nc.semaphore(f"recv_{p}")) for p in range(2)]
        send = [E(nc.semaphore(f"send_{p}")) for p in range(2)]
        prep_sem = E(nc.semaphore("prep_sem"))
        pe_z = E(nc.semaphore("pe_z"))
        th_sem = E(nc.semaphore("th_sem"))
        tr_sem = E(nc.semaphore("tr_sem"))
        ctr_sem = E(nc.semaphore("ctr_sem"))
        hst_sem = E(nc.semaphore("hst_sem"))
        tld_sem = E(nc.semaphore("tld_sem"))
        pro_sem = E(nc.semaphore("pro_sem"))
        oc_sem = E(nc.semaphore("oc_sem"))
        od_sem = E(nc.semaphore("od_sem"))

        wht_sb = E(nc.sbuf_tensor("wht_sb", [128, NK * HS], bf16))
        wihA_sb = E(nc.sbuf_tensor("wihA_sb", [65, HS], bf16))
        wihB_sb = E(nc.sbuf_tensor("wihB_sb", [64, HS], bf16))
        wout_sb = E(nc.sbuf_tensor("wout_sb", [128, NK * OUT_TOT], bf16))
        bout_sb = E(nc.sbuf_tensor("bout_sb", [OUT_TOT, 1], f32))
        ident_sb = E(nc.sbuf_tensor("ident_sb", [128, 128], bf16))
        id_sb = E(nc.sbuf_tensor("id_sb", [1, 1], mybir.dt.uint32))
        tblk_sb = E(nc.sbuf_tensor("tblk_sb", [1, NROUND], mybir.dt.uint32))
        xA_sb = E(nc.sbuf_tensor("xA_sb", [65, 2 * XB * 128], bf16))
        xB_sb = E(nc.sbuf_tensor("xB_sb", [64, 2 * XB * 128], bf16))
        hall0 = E(nc.sbuf_tensor("hall0", [128, H], bf16))
        hall1 = E(nc.sbuf_tensor("hall1", [128, H], bf16))
        h_sb = E(nc.sbuf_tensor("h_sb", [128, 2 * HS], bf16))
        hmine = E(nc.sbuf_tensor("hmine", [128, 2 * HS], bf16))
        hload = E(nc.sbuf_tensor("hload", [128, XB * H], bf16))
        ostage = E(nc.sbuf_tensor("ostage", [OUT_TOT, 512], f32))

        pz = [E(nc.psum_tensor(f"pz{p}", [128, HS], f32)) for p in range(2)]
        ptr = [E(nc.psum_tensor(f"ptr{p}", [128, HS], bf16)) for p in range(2)]
        pro = E(nc.psum_tensor("pro", [OUT_TOT, 512], f32))

        hall = [hall0, hall1]
        N_INIT = 16 * 9  # init incs into w_sem

        with nc.Block() as block:

            @block.gpsimd
            def _(gp):
                gp.dma_start(out=wht_sb[:, :], in_=wht[:, :]).then_inc(w_sem, 16)
                gp.dma_start(out=wihA_sb[:, :], in_=wihA[:, :]).then_inc(w_sem, 16)
                gp.dma_start(out=wihB_sb[:, :], in_=wihB[:, :]).then_inc(w_sem, 16)
                gp.dma_start(out=wout_sb[:, :], in_=wout[:, :]).then_inc(w_sem, 16)
                gp.dma_start(out=bout_sb[:, :], in_=bout[:, :]).then_inc(w_sem, 16)
                gp.dma_start(out=ident_sb[:, :], in_=ident[:, :]).then_inc(w_sem, 16)
                gp.dma_start(out=id_sb[:, :], in_=myoff[:, :]).then_inc(w_sem, 16)
                gp.dma_start(out=tblk_sb[:, :], in_=tblk[:, :]).then_inc(w_sem, 16)
                gp.memset(hall1[:, :], 0).then_inc(recv[1], 16)
                gp.memset(xA_sb[64:65, :], 1.0).then_inc(w_sem, 16)
                gp.wait_ge(w_sem, N_INIT)
                with gp.register("pidr") as pidr:
                    gp.reg_load(pidr, id_sb[0:1, 0:1])
                    off = gp.snap(pidr, min_val=0, max_val=H - HS)
                rdests = [(0, d) for d in range(8)]
                for t in range(T):
                    p = t & 1
                    gp.remote_dma_broadcast(
                        out_ap=hall[p][:, bass.ds(off, HS)],
                        in_ap=hmine[:, p * HS:p * HS + HS],
                        remote_sem=recv[p],
                        local_sem=send[p],
                        rdests=rdests,
                    ).then_inc(prep_sem, 1)
                    gp.wait_ge(prep_sem, t + 1)
                    gp.wait_ge(ctr_sem, t + 1)
                    if t > 0:
                        gp.wait_ge(hst_sem, 16 * t)
                    gp.trigger_dma(count=1)
                # tail: load h blocks (register-offset DMAs)
                gp.wait_ge(hst_sem, 16 * T)
                with gp.register("tb") as tb:
                    for r in range(NROUND):
                        gp.reg_load(tb, tblk_sb[0:1, r:r + 1])
                        t0 = gp.snap(tb, min_val=0, max_val=t_io - XB)
                        if r > 0:
                            gp.wait_ge(pro_sem, r)
                        gp.dma_start(
                            out=hload[:, :].rearrange("p (s f) -> p s f", s=XB),
                            in_=hstore[bass.ds(t0, XB)].rearrange(
                                "s p f -> p s f"),
                        ).then_inc(tld_sem, 16)

            @block.sync
            def _(sy):
                def x_dma(q):
                    qs = q % nxb_io
                    slot = (q % 2) * XB * 128
                    sy.dma_start(
                        out=xA_sb[0:64, slot:slot + XB * 128],
                        in_=xt[qs][0:64, :],
                    ).then_inc(x_sem, 16)
                    sy.dma_start(
                        out=xB_sb[0:64, slot:slot + XB * 128],
                        in_=xt[qs][64:128, :],
                    ).then_inc(x_sem, 16)

                x_dma(0)
                x_dma(1)
                for t in range(T):
                    p = t & 1
                    # store h_t (complete in hall[p] after all arrivals)
                    sy.wait_ge(recv[p], 16 * (t // 2 + 1 + p))
                    sy.dma_start(out=hstore[t % t_io],
                                 in_=hall[p][:, :]).then_inc(hst_sem, 16)
                    if t % XB == 3 and t // XB + 2 < nxb:
                        sy.wait_ge(pe_z, t + 1)
                        x_dma(t // XB + 2)
                # tail: ship results
                for r in range(NROUND):
                    sy.wait_ge(oc_sem, r + 1)
                    sy.dma_start(out=out_tail[r],
                                 in_=ostage[:, :]).then_inc(od_sem, 16)

            @block.tensor
            def _(pe):
                pe.wait_ge(w_sem, N_INIT)
                for t in range(T):
                    p = t & 1
                    q = 1 - p
                    slot = (t % (2 * XB)) * 128
                    # openers (input proj + bias) for step t
                    pe.wait_ge(x_sem, 32 * (t // XB + 1))
                    if t >= 1:
                        pe.wait_ge(th_sem, t)  # pz[p] free + h_sb[q] ready
                    pe.matmul(pz[p][:, :], xA_sb[0:65, slot:slot + 128],
                              wihA_sb[0:65, :], start=True, stop=False)
                    pe.matmul(pz[p][:, :], xB_sb[0:64, slot:slot + 128],
                              wihB_sb[0:64, :], start=False, stop=False)
                    # transposes for step t-1 (hide under scalar tanh(t-1))
                    if t >= 1:
                        if t >= 3:
                            pe.wait_ge(ctr_sem, t - 2)  # ptr[q] free
                        pe.transpose(ptr[q][:, 0:128],
                                     h_sb[:, q * HS:q * HS + 128], ident_sb[:, :])
                        pe.transpose(ptr[q][:, 128:256],
                                     h_sb[:, q * HS + 128:q * HS + 256],
                                     ident_sb[:, :]).then_inc(tr_sem, 1)
                    # recurrent matmuls for step t
                    pe.wait_ge(recv[q], 16 * (t // 2 + 1))
                    for j in range(NK):
                        ins = pe.matmul(
                            pz[p][:, :], hall[q][:, 128 * j:128 * (j + 1)],
                            wht_sb[:, HS * j:HS * (j + 1)],
                            start=False, stop=(j == NK - 1),
                        )
                        if j == NK - 1:
                            ins.then_inc(pe_z, 1)
                # final transpose for t = T-1
                pf = (T - 1) & 1
                pe.wait_ge(th_sem, T)
                pe.wait_ge(ctr_sem, T - 2)
                pe.transpose(ptr[pf][:, 0:128],
                             h_sb[:, pf * HS:pf * HS + 128], ident_sb[:, :])
                pe.transpose(ptr[pf][:, 128:256],
                             h_sb[:, pf * HS + 128:pf * HS + 256],
                             ident_sb[:, :]).then_inc(tr_sem, 1)
                # tail readout: blocks of 4 steps, N=512
                hl = hload[:, :].rearrange("p (s f) -> p s f", s=XB)
                for r in range(NROUND):
                    pe.wait_ge(tld_sem, 16 * (r + 1))
                    if r > 0:
                        pe.wait_ge(oc_sem, r)  # pro free
                    for k in range(NK):
                        ins = pe.matmul(
                            pro[:, :], wout_sb[:, OUT_TOT * k:OUT_TOT * (k + 1)],
                            hl[:, :, 128 * k:128 * (k + 1)],
                            start=(k == 0), stop=(k == NK - 1),
                        )
                        if k == NK - 1:
                            ins.then_inc(pro_sem, 1)

            @block.scalar
            def _(act):
                for t in range(T):
                    p = t & 1
                    act.wait_ge(pe_z, t + 1)
                    if t >= 2:
                        act.wait_ge(tr_sem, t - 1)  # h_sb[p] free
                    act.activation(
                        out=h_sb[:, p * HS:p * HS + HS],
                        in_=pz[p][:, :],
                        func=mybir.ActivationFunctionType.Tanh,
                    ).then_inc(th_sem, 1)
                    act.wait_ge(tr_sem, t + 1)  # ptr[p] written
                    if t >= 2:
                        act.wait_ge(send[p], 16 * (t // 2))  # hmine[p] free
                    act.activation(
                        out=hmine[:, p * HS:p * HS + HS],
                        in_=ptr[p][:, :],
                        func=mybir.ActivationFunctionType.Copy,
                    ).then_inc(ctr_sem, 1)
                # tail: add b_out, stage for DMA
                for r in range(NROUND):
                    act.wait_ge(pro_sem, r + 1)
                    if r > 0:
                        act.wait_ge(od_sem, 16 * r)  # ostage free
                    act.activation(
                        out=ostage[:, :],
                        in_=pro[:, :],
                        func=mybir.ActivationFunctionType.Identity,
                        bias=bout_sb[:, 0:1],
                    ).then_inc(oc_sem, 1)

    return nc


def prep_in_maps(x, w_ih, b_ih, w_hh, b_hh, w_out, b_out,
                 input_mask, hh_mask, out_mask, T=T_FULL):
    f32 = np.float32
    Wih = np.asarray(w_ih, f32) * np.asarray(input_mask, f32)
    Whh = np.asarray(w_hh, f32) * np.asarray(hh_mask, f32)
    Wout = np.asarray(w_out, f32) * np.asarray(out_mask, f32)
    bsum = np.asarray(b_ih, f32) + np.asarray(b_hh, f32)
    bo = np.asarray(b_out, f32)

    x = np.asarray(x, f32)[:T]
    nxb = T // XB

    in_maps = []
    for c in range(NCORES):
        r0 = HS * c
        i_mod = c // 2
        # recurrent moving tiles: wht[k, 256j + ho] = Whh[r0+ho, 128j+k]
        wht = np.empty((128, NK, HS), BF16)
        for j in range(NK):
            wht[:, j, :] = Whh[r0:r0 + HS, 128 * j:128 * (j + 1)].T
        # opener moving: rows = x indices within module chunk, + bias row (A)
        xc0 = 128 * i_mod
        wihA = np.empty((65, HS), BF16)
        wihA[0:64] = Wih[r0:r0 + HS, xc0:xc0 + 64].T
        wihA[64] = bsum[r0:r0 + HS]
        wihB = np.ascontiguousarray(
            Wih[r0:r0 + HS, xc0 + 64:xc0 + 128].T).astype(BF16)
        # readout stationary tiles: wout[k, 64kk + o] = Wout[o, 128kk+k]
        wo = np.empty((128, NK, OUT_TOT), BF16)
        for k in range(NK):
            wo[:, k, :] = Wout[:, 128 * k:128 * (k + 1)].T
        # x batches: xt[q, r, 128*dt + b] = x[4q+dt, b, 128*i_mod + r]
        xq = x[:, :, xc0:xc0 + 128]            # [T, B, 128]
        xq = xq.reshape(nxb, XB, B, 128)        # [q, dt, b, r]
        xq = xq.transpose(0, 3, 1, 2)           # [q, r, dt, b]
        xtc = np.ascontiguousarray(xq.reshape(nxb, 128, XB * B)).astype(BF16)
        # tail block starts
        tb = np.array([[4 * min(c + 8 * r, NBLK - 1) for r in range(NROUND)]],
                      np.uint32)
        in_maps.append({
            "wht": np.ascontiguousarray(wht.reshape(128, NK * HS)),
            "wihA": np.ascontiguousarray(wihA),
            "wihB": wihB,
            "wout": np.ascontiguousarray(wo.reshape(128, NK * OUT_TOT)),
            "bout": np.ascontiguousarray(bo.reshape(OUT_TOT, 1)),
            "ident": np.eye(128, dtype=BF16),
            "myoff": np.array([[HS * c]], np.uint32),
            "tblk": tb,
            "xt": xtc,
        })
    return in_maps


def assemble(results, T=T_FULL):
    out = np.empty((T, 4, B, 16), np.float32)
    for blk in range(NBLK):
        c = blk % 8 if blk < NBLK - 1 else 0
        r = blk // 8 if blk < NBLK - 1 else NROUND - 1
        stage = results[c]["out_tail"][r]          # [64, 512]
        acc = stage.reshape(OUT_TOT, XB, B)        # [o, dt, b]
        for dt in range(XB):
            t = XB * blk + dt
            if t < T:
                out[t] = acc[:, dt, :].reshape(4, 16, B).transpose(0, 2, 1)
    return out


_CACHE = {}
_LAST_RESULT = None


def kernel(**inputs) -> np.ndarray:
    global _LAST_RESULT
    import os
    T = inputs["x"].shape[0]
    if T not in _CACHE:
        nc = build(T)
        nc.finalize()
        _CACHE[T] = nc
    nc = _CACHE[T]
    in_maps = prep_in_maps(T=T, **inputs)
    trace = bool(int(os.environ.get("BASS_RNN_TRACE", "0")))
    res = run_bass_kernel_spmd(
        nc, in_maps, core_ids=list(range(NCORES)), trace=trace
    )
    _LAST_RESULT = res
    return assemble(res.results, T=T)

